# revision 6
# baseline (speedup 1.0000x reference)
"""Trainium2 Bass kernel for ConceptHierarchyModule (GNN message passing).

Reference computation (per valid edge lo->hi, valid = 0<=ll<4 and hl>ll):
    out = node_features + segment_sum(0.2*(x[lo] @ W[ll].T + b[ll]), hi)

Strategy (8 cores, destination-sharded, zero collectives):
  - Host drops invalid edges (~60%), assigns each edge to the core owning its
    destination node (12500 dests/core), groups edges by 128-dest tile, and
    encodes rel = level*128 + (dest % 128) in [0, 512).
  - Device, per dest-tile ("slot"): indirect-gather source rows x[lo] from the
    full replicated X table in HBM; build one-hot M[e, rel] via is_equal
    against an iota row; accumulate U[f, l*128+d] += Xg.T @ M on the PE
    (PSUM); then out[f', d] = sum_l (0.2*W[l].T).T @ U_l + (0.2*b).T @ counts
    + X[dest].T  -- all matmuls, linearity moves the W transform after the
    destination segment-sum so it runs once per dest, not per edge.
  - Per-core program structure (slot subchunk counts) is made identical across
    cores by rank-sorting slots by edge count and padding to the per-rank max,
    so one SPMD NEFF serves all 8 cores; all data differences live in inputs.
  - Output is [128, 12544] feature-major per core; host transposes/unpermutes.
"""

import numpy as np

import concourse.bass as bass
import concourse.mybir as mybir
import concourse.tile as tile
from concourse import bacc
from concourse.bass_utils import run_bass_kernel_spmd

N, F, LEVELS, E = 100000, 128, 4, 500000
NC = 8
NPC = N // NC  # 12500 destination nodes per core
TILES = (NPC + 127) // 128  # 98 dest tiles per core
NPAD = TILES * 128  # 12544
P = 128
MCOL = LEVELS * P  # 512 one-hot columns (level, dest-in-tile)
GROUP = 16  # gather subchunks per indirect DMA

F32 = mybir.dt.float32
I32 = mybir.dt.int32

# Results of the last kernel() call, for the local test harness.
LAST_RESULTS = None


def _plan_core(lo, hi, ll, c):
    """Per-core edge layout: returns (tile_id-sorted arrays, counts per tile)."""
    dloc = hi - c * NPC
    tid = dloc // P
    d = dloc % P
    rel = ll * P + d
    order = np.argsort(tid, kind="stable")
    return tid[order], lo[order], rel[order], d[order], np.bincount(
        tid, minlength=TILES
    )


def _make_plan(node_features, W, b, hierarchy_levels, hierarchy_edges):
    lo = hierarchy_edges[:, 0].astype(np.int64)
    hi = hierarchy_edges[:, 1].astype(np.int64)
    ll = hierarchy_levels[lo].astype(np.int64)
    hl = hierarchy_levels[hi].astype(np.int64)
    valid = (ll >= 0) & (ll < LEVELS) & (hl > ll)
    lo, hi, ll = lo[valid], hi[valid], ll[valid]
    core = hi // NPC

    per_core = []
    for c in range(NC):
        m = core == c
        per_core.append(_plan_core(lo[m], hi[m], ll[m], c))

    # ks[c, t] = subchunks needed by core c's t-th tile; rank-sort descending.
    ks = np.stack([np.ceil(pc[4] / P).astype(np.int64) for pc in per_core])
    tile_order = [np.argsort(-pc[4], kind="stable") for pc in per_core]
    ks_sorted = np.stack(
        [ks[c][tile_order[c]] for c in range(NC)]
    )  # [NC, TILES] descending
    K_list = ks_sorted.max(axis=0)  # shared per-slot subchunk count
    sub_base = np.concatenate([[0], np.cumsum(K_list)])
    S_tot = int(sub_base[-1])

    cores = []
    for c in range(NC):
        tid_s, lo_s, rel_s, d_s, counts = per_core[c]
        order = tile_order[c]  # slot s -> original tile id
        slot_of_tile = np.empty(TILES, dtype=np.int64)
        slot_of_tile[order] = np.arange(TILES)

        # Edge positions in the padded [S_tot, 128] layout.
        tile_start = np.concatenate([[0], np.cumsum(counts)])
        idx_within = np.arange(len(tid_s)) - tile_start[tid_s]
        slot_e = slot_of_tile[tid_s]
        sub = sub_base[slot_e] + idx_within // P
        part = idx_within % P

        src = np.zeros((S_tot, P), dtype=np.int32)
        relp = np.full((S_tot, P), -1.0, dtype=np.float32)
        src[sub, part] = lo_s.astype(np.int32)
        relp[sub, part] = rel_s.astype(np.float32)

        # Per-level counts per dest, in slot order: cnt[l, s*128 + d].
        col = slot_e * 0  # placeholder; compute via level and dest-in-tile
        lvl_e = rel_s // P
        col = slot_e * P + (rel_s % P)
        cnt = np.bincount(
            lvl_e * NPAD + col, minlength=LEVELS * NPAD
        ).reshape(LEVELS, NPAD).astype(np.float32)

        # Node id per output column (slot order); -1 for padding columns.
        t_of_s = order
        node_of_col = (
            c * NPC + (t_of_s[:, None] * P + np.arange(P)[None, :])
        ).reshape(-1)
        node_of_col[node_of_col >= (c + 1) * NPC] = -1

        xdt = np.zeros((P, NPAD), dtype=np.float32)
        real = node_of_col >= 0
        xdt[:, real] = node_features[node_of_col[real]].T

        cores.append(
            dict(
                srct=np.ascontiguousarray(src.T),  # [128, S_tot] int32
                relt=np.ascontiguousarray(relp.T),  # [128, S_tot] f32
                cnt=cnt,
                xdt=xdt,
                node_of_col=node_of_col,
            )
        )

    return K_list.tolist(), S_tot, cores


def _build_program(K_list, S_tot):
    nc = bacc.Bacc("TRN2", target_bir_lowering=False)

    x = nc.dram_tensor("x", [N, F], F32, kind="ExternalInput")
    srct = nc.dram_tensor("srct", [P, S_tot], I32, kind="ExternalInput")
    relt = nc.dram_tensor("relt", [P, S_tot], F32, kind="ExternalInput")
    cnt = nc.dram_tensor("cnt", [LEVELS, NPAD], F32, kind="ExternalInput")
    xdt = nc.dram_tensor("xdt", [P, NPAD], F32, kind="ExternalInput")
    wt = nc.dram_tensor("wt", [F, MCOL], F32, kind="ExternalInput")
    bb = nc.dram_tensor("bb", [LEVELS, F], F32, kind="ExternalInput")
    iota = nc.dram_tensor("iota", [P, MCOL], F32, kind="ExternalInput")
    outt = nc.dram_tensor("outt", [P, NPAD], F32, kind="ExternalOutput")

    sub_base = [0]
    for k in K_list:
        sub_base.append(sub_base[-1] + k)

    with tile.TileContext(nc) as tc:
        with (
            tc.tile_pool(name="const", bufs=1) as constp,
            tc.tile_pool(name="gat", bufs=8) as gatp,
            tc.tile_pool(name="m", bufs=4) as mp,
            tc.tile_pool(name="u", bufs=3) as up,
            tc.tile_pool(name="io", bufs=4) as iop,
            tc.tile_pool(name="psA", bufs=2, space="PSUM") as psA,
            tc.tile_pool(name="psB", bufs=4, space="PSUM") as psB,
        ):
            iota_sb = constp.tile([P, MCOL], F32)
            nc.sync.dma_start(out=iota_sb[:], in_=iota[:])
            wt_sb = constp.tile([F, MCOL], F32)
            nc.sync.dma_start(out=wt_sb[:], in_=wt[:])
            bb_sb = constp.tile([LEVELS, F], F32)
            nc.sync.dma_start(out=bb_sb[:], in_=bb[:])
            cnt_sb = constp.tile([LEVELS, NPAD], F32)
            nc.sync.dma_start(out=cnt_sb[:], in_=cnt[:])
            srct_sb = constp.tile([P, S_tot], I32)
            nc.sync.dma_start(out=srct_sb[:], in_=srct[:])
            relt_sb = constp.tile([P, S_tot], F32)
            nc.sync.dma_start(out=relt_sb[:], in_=relt[:])

            for s, K in enumerate(K_list):
                if K > 0:
                    psumA = psA.tile([P, MCOL], F32)
                    for j in range(K):
                        gidx = sub_base[s] + j
                        # HW indirect gather consumes exactly one index per
                        # partition (one descriptor each): 128 rows per DMA.
                        xg = gatp.tile([P, F], F32)
                        nc.gpsimd.indirect_dma_start(
                            out=xg[:],
                            out_offset=None,
                            in_=x[:],
                            in_offset=bass.IndirectOffsetOnAxis(
                                ap=srct_sb[:, gidx : gidx + 1], axis=0
                            ),
                        )
                        mt = mp.tile([P, MCOL], F32)
                        nc.vector.tensor_tensor(
                            out=mt[:],
                            in0=relt_sb[:, gidx : gidx + 1].to_broadcast([P, MCOL]),
                            in1=iota_sb[:],
                            op=mybir.AluOpType.is_equal,
                        )
                        nc.tensor.matmul(
                            out=psumA[:],
                            lhsT=xg[:],
                            rhs=mt[:],
                            start=(j == 0),
                            stop=(j == K - 1),
                        )
                    usb = up.tile([P, MCOL], F32)
                    nc.scalar.copy(out=usb[:], in_=psumA[:])

                psumB = psB.tile([P, P], F32)
                nc.tensor.matmul(
                    out=psumB[:],
                    lhsT=bb_sb[:],
                    rhs=cnt_sb[:, s * P : (s + 1) * P],
                    start=True,
                    stop=(K == 0),
                )
                if K > 0:
                    for l in range(LEVELS):
                        nc.tensor.matmul(
                            out=psumB[:],
                            lhsT=wt_sb[:, l * P : (l + 1) * P],
                            rhs=usb[:, l * P : (l + 1) * P],
                            start=False,
                            stop=(l == LEVELS - 1),
                        )
                xdt_t = iop.tile([P, P], F32)
                nc.sync.dma_start(out=xdt_t[:], in_=xdt[:, s * P : (s + 1) * P])
                ot = iop.tile([P, P], F32)
                nc.vector.tensor_add(out=ot[:], in0=xdt_t[:], in1=psumB[:])
                nc.sync.dma_start(out=outt[:, s * P : (s + 1) * P], in_=ot[:])

    nc.compile()
    return nc


def kernel(
    node_features, W, b, hierarchy_levels, hierarchy_edges, _trace=False
):
    global LAST_RESULTS
    node_features = np.asarray(node_features, dtype=np.float32)
    W = np.asarray(W, dtype=np.float32)
    b = np.asarray(b, dtype=np.float32)
    hierarchy_levels = np.asarray(hierarchy_levels, dtype=np.int32)
    hierarchy_edges = np.asarray(hierarchy_edges, dtype=np.int32)

    K_list, S_tot, cores = _make_plan(
        node_features, W, b, hierarchy_levels, hierarchy_edges
    )

    wt = np.ascontiguousarray(
        (0.2 * np.transpose(W, (2, 0, 1))).reshape(F, MCOL)
    ).astype(np.float32)
    bb = (0.2 * b).astype(np.float32)
    iota = np.broadcast_to(
        np.arange(MCOL, dtype=np.float32)[None, :], (P, MCOL)
    ).copy()

    nc = _build_program(K_list, S_tot)

    in_maps = []
    for c in range(NC):
        pc = cores[c]
        in_maps.append(
            dict(
                x=node_features,
                srct=pc["srct"],
                relt=pc["relt"],
                cnt=pc["cnt"],
                xdt=pc["xdt"],
                wt=wt,
                bb=bb,
                iota=iota,
            )
        )

    # trace=True needs the axon NTFF hook (antenv.axon_hooks), absent in this
    # environment -- always run untraced.
    res = run_bass_kernel_spmd(nc, in_maps, core_ids=list(range(NC)))
    LAST_RESULTS = res

    out = np.empty((N, F), dtype=np.float32)
    for c in range(NC):
        o = res.results[c]["outt"]  # [128, NPAD]
        noc = cores[c]["node_of_col"]
        real = noc >= 0
        out[noc[real]] = o[:, real].T
    return out


# revision 9
# speedup vs baseline: 202.2588x; 202.2588x over previous
"""Trainium2 Bass kernel for ConceptHierarchyModule (GNN message passing).

Reference computation (per valid edge lo->hi, valid = 0<=ll<4 and hl>ll):
    out = node_features + segment_sum(0.2*(x[lo] @ W[ll].T + b[ll]), hi)

Strategy (8 cores, destination-sharded, zero collectives):
  - Host drops invalid edges (~60%), assigns each edge to the core owning its
    destination node (12500 dests/core), groups edges by 128-dest tile, and
    encodes rel = level*128 + (dest % 128) in [0, 512).
  - Device, per dest-tile ("slot"): indirect-gather source rows x[lo] from the
    full replicated X table in HBM; build one-hot M[e, rel] via is_equal
    against an iota row; accumulate U[f, l*128+d] += Xg.T @ M on the PE
    (PSUM); then out[f', d] = sum_l (0.2*W[l].T).T @ U_l + (0.2*b).T @ counts
    + X[dest].T  -- all matmuls, linearity moves the W transform after the
    destination segment-sum so it runs once per dest, not per edge.
  - Per-core program structure (slot subchunk counts) is made identical across
    cores by rank-sorting slots by edge count and padding to the per-rank max,
    so one SPMD NEFF serves all 8 cores; all data differences live in inputs.
  - Output is [128, 12544] feature-major per core; host transposes/unpermutes.
"""

import numpy as np

import concourse.bass as bass
import concourse.mybir as mybir
import concourse.tile as tile
from concourse import bacc
from concourse.bass_utils import run_bass_kernel_spmd

N, F, LEVELS, E = 100000, 128, 4, 500000
NC = 8
NPC = N // NC  # 12500 destination nodes per core
TILES = (NPC + 127) // 128  # 98 dest tiles per core
NPAD = TILES * 128  # 12544
P = 128
MCOL = LEVELS * P  # 512 one-hot columns (level, dest-in-tile)
GROUP = 16  # gather subchunks per indirect DMA

F32 = mybir.dt.float32
I32 = mybir.dt.int32

# Results of the last kernel() call, for the local test harness.
LAST_RESULTS = None


def _plan_core(lo, hi, ll, c):
    """Per-core edge layout: returns (tile_id-sorted arrays, counts per tile)."""
    dloc = hi - c * NPC
    tid = dloc // P
    d = dloc % P
    rel = ll * P + d
    order = np.argsort(tid, kind="stable")
    return tid[order], lo[order], rel[order], d[order], np.bincount(
        tid, minlength=TILES
    )


def _make_plan(node_features, W, b, hierarchy_levels, hierarchy_edges):
    lo = hierarchy_edges[:, 0].astype(np.int64)
    hi = hierarchy_edges[:, 1].astype(np.int64)
    ll = hierarchy_levels[lo].astype(np.int64)
    hl = hierarchy_levels[hi].astype(np.int64)
    valid = (ll >= 0) & (ll < LEVELS) & (hl > ll)
    lo, hi, ll = lo[valid], hi[valid], ll[valid]
    core = hi // NPC

    per_core = []
    for c in range(NC):
        m = core == c
        per_core.append(_plan_core(lo[m], hi[m], ll[m], c))

    # ks[c, t] = subchunks needed by core c's t-th tile; rank-sort descending.
    ks = np.stack([np.ceil(pc[4] / P).astype(np.int64) for pc in per_core])
    tile_order = [np.argsort(-pc[4], kind="stable") for pc in per_core]
    ks_sorted = np.stack(
        [ks[c][tile_order[c]] for c in range(NC)]
    )  # [NC, TILES] descending
    K_list = ks_sorted.max(axis=0)  # shared per-slot subchunk count
    sub_base = np.concatenate([[0], np.cumsum(K_list)])
    S_tot = int(sub_base[-1])

    cores = []
    for c in range(NC):
        tid_s, lo_s, rel_s, d_s, counts = per_core[c]
        order = tile_order[c]  # slot s -> original tile id
        slot_of_tile = np.empty(TILES, dtype=np.int64)
        slot_of_tile[order] = np.arange(TILES)

        # Edge positions in the padded [S_tot, 128] layout.
        tile_start = np.concatenate([[0], np.cumsum(counts)])
        idx_within = np.arange(len(tid_s)) - tile_start[tid_s]
        slot_e = slot_of_tile[tid_s]
        sub = sub_base[slot_e] + idx_within // P
        part = idx_within % P

        src = np.zeros((S_tot, P), dtype=np.int32)
        relp = np.full((S_tot, P), -1.0, dtype=np.float32)
        src[sub, part] = lo_s.astype(np.int32)
        relp[sub, part] = rel_s.astype(np.float32)

        # Per-level counts per dest, in slot order: cnt[l, s*128 + d].
        col = slot_e * 0  # placeholder; compute via level and dest-in-tile
        lvl_e = rel_s // P
        col = slot_e * P + (rel_s % P)
        cnt = np.bincount(
            lvl_e * NPAD + col, minlength=LEVELS * NPAD
        ).reshape(LEVELS, NPAD).astype(np.float32)

        # Node id per output column (slot order); -1 for padding columns.
        t_of_s = order
        node_of_col = (
            c * NPC + (t_of_s[:, None] * P + np.arange(P)[None, :])
        ).reshape(-1)
        node_of_col[node_of_col >= (c + 1) * NPC] = -1

        xdt = np.zeros((P, NPAD), dtype=np.float32)
        real = node_of_col >= 0
        xdt[:, real] = node_features[node_of_col[real]].T

        cores.append(
            dict(
                srct=np.ascontiguousarray(src.T),  # [128, S_tot] int32
                relt=np.ascontiguousarray(relp.T),  # [128, S_tot] f32
                cnt=cnt,
                xdt=xdt,
                node_of_col=node_of_col,
            )
        )

    return K_list.tolist(), S_tot, cores


def _build_program(K_list, S_tot, repeat=1):
    """Build the SPMD program. repeat>1 wraps the whole body in a hardware
    loop (used only for timing: marginal iteration cost isolates NEFF exec
    time from the per-call axon dispatch overhead)."""
    nc = bacc.Bacc("TRN2", target_bir_lowering=False)

    x = nc.dram_tensor("x", [N, F], F32, kind="ExternalInput")
    srct = nc.dram_tensor("srct", [P, S_tot], I32, kind="ExternalInput")
    relt = nc.dram_tensor("relt", [P, S_tot], F32, kind="ExternalInput")
    cnt = nc.dram_tensor("cnt", [LEVELS, NPAD], F32, kind="ExternalInput")
    xdt = nc.dram_tensor("xdt", [P, NPAD], F32, kind="ExternalInput")
    wt = nc.dram_tensor("wt", [F, MCOL], F32, kind="ExternalInput")
    bb = nc.dram_tensor("bb", [LEVELS, F], F32, kind="ExternalInput")
    iota = nc.dram_tensor("iota", [P, MCOL], F32, kind="ExternalInput")
    outt = nc.dram_tensor("outt", [P, NPAD], F32, kind="ExternalOutput")

    sub_base = [0]
    for k in K_list:
        sub_base.append(sub_base[-1] + k)

    with tile.TileContext(nc) as tc:
        with (
            tc.tile_pool(name="const", bufs=1) as constp,
            tc.tile_pool(name="gat", bufs=8) as gatp,
            tc.tile_pool(name="m", bufs=4) as mp,
            tc.tile_pool(name="u", bufs=3) as up,
            tc.tile_pool(name="io", bufs=4) as iop,
            tc.tile_pool(name="psA", bufs=2, space="PSUM") as psA,
            tc.tile_pool(name="psB", bufs=4, space="PSUM") as psB,
        ):
            iota_sb = constp.tile([P, MCOL], F32)
            nc.sync.dma_start(out=iota_sb[:], in_=iota[:])
            wt_sb = constp.tile([F, MCOL], F32)
            nc.sync.dma_start(out=wt_sb[:], in_=wt[:])
            bb_sb = constp.tile([LEVELS, F], F32)
            nc.sync.dma_start(out=bb_sb[:], in_=bb[:])
            cnt_sb = constp.tile([LEVELS, NPAD], F32)
            nc.sync.dma_start(out=cnt_sb[:], in_=cnt[:])
            srct_sb = constp.tile([P, S_tot], I32)
            nc.sync.dma_start(out=srct_sb[:], in_=srct[:])
            relt_sb = constp.tile([P, S_tot], F32)
            nc.sync.dma_start(out=relt_sb[:], in_=relt[:])

            def emit_body():
              for s, K in enumerate(K_list):
                if K > 0:
                    psumA = psA.tile([P, MCOL], F32)
                    for j in range(K):
                        gidx = sub_base[s] + j
                        # HW indirect gather consumes exactly one index per
                        # partition (one descriptor each): 128 rows per DMA.
                        xg = gatp.tile([P, F], F32)
                        nc.gpsimd.indirect_dma_start(
                            out=xg[:],
                            out_offset=None,
                            in_=x[:],
                            in_offset=bass.IndirectOffsetOnAxis(
                                ap=srct_sb[:, gidx : gidx + 1], axis=0
                            ),
                        )
                        mt = mp.tile([P, MCOL], F32)
                        nc.vector.tensor_tensor(
                            out=mt[:],
                            in0=relt_sb[:, gidx : gidx + 1].to_broadcast([P, MCOL]),
                            in1=iota_sb[:],
                            op=mybir.AluOpType.is_equal,
                        )
                        nc.tensor.matmul(
                            out=psumA[:],
                            lhsT=xg[:],
                            rhs=mt[:],
                            start=(j == 0),
                            stop=(j == K - 1),
                        )
                    usb = up.tile([P, MCOL], F32)
                    nc.scalar.copy(out=usb[:], in_=psumA[:])

                psumB = psB.tile([P, P], F32)
                nc.tensor.matmul(
                    out=psumB[:],
                    lhsT=bb_sb[:],
                    rhs=cnt_sb[:, s * P : (s + 1) * P],
                    start=True,
                    stop=(K == 0),
                )
                if K > 0:
                    for l in range(LEVELS):
                        nc.tensor.matmul(
                            out=psumB[:],
                            lhsT=wt_sb[:, l * P : (l + 1) * P],
                            rhs=usb[:, l * P : (l + 1) * P],
                            start=False,
                            stop=(l == LEVELS - 1),
                        )
                xdt_t = iop.tile([P, P], F32)
                nc.sync.dma_start(out=xdt_t[:], in_=xdt[:, s * P : (s + 1) * P])
                ot = iop.tile([P, P], F32)
                nc.vector.tensor_add(out=ot[:], in0=xdt_t[:], in1=psumB[:])
                nc.sync.dma_start(out=outt[:, s * P : (s + 1) * P], in_=ot[:])

            if repeat > 1:
                with tc.For_i(0, repeat, 1):
                    emit_body()
            else:
                emit_body()

    nc.compile()
    return nc


def kernel(
    node_features, W, b, hierarchy_levels, hierarchy_edges, _trace=False
):
    global LAST_RESULTS
    node_features = np.asarray(node_features, dtype=np.float32)
    W = np.asarray(W, dtype=np.float32)
    b = np.asarray(b, dtype=np.float32)
    hierarchy_levels = np.asarray(hierarchy_levels, dtype=np.int32)
    hierarchy_edges = np.asarray(hierarchy_edges, dtype=np.int32)

    K_list, S_tot, cores = _make_plan(
        node_features, W, b, hierarchy_levels, hierarchy_edges
    )

    wt = np.ascontiguousarray(
        (0.2 * np.transpose(W, (2, 0, 1))).reshape(F, MCOL)
    ).astype(np.float32)
    bb = (0.2 * b).astype(np.float32)
    iota = np.broadcast_to(
        np.arange(MCOL, dtype=np.float32)[None, :], (P, MCOL)
    ).copy()

    nc = _build_program(K_list, S_tot)

    in_maps = []
    for c in range(NC):
        pc = cores[c]
        in_maps.append(
            dict(
                x=node_features,
                srct=pc["srct"],
                relt=pc["relt"],
                cnt=pc["cnt"],
                xdt=pc["xdt"],
                wt=wt,
                bb=bb,
                iota=iota,
            )
        )

    # trace=True needs the axon NTFF hook (antenv.axon_hooks), absent in this
    # environment -- always run untraced.
    res = run_bass_kernel_spmd(nc, in_maps, core_ids=list(range(NC)))
    LAST_RESULTS = res

    out = np.empty((N, F), dtype=np.float32)
    for c in range(NC):
        o = res.results[c]["outt"]  # [128, NPAD]
        noc = cores[c]["node_of_col"]
        real = noc >= 0
        out[noc[real]] = o[:, real].T
    return out
